# revision 10
# baseline (speedup 1.0000x reference)
"""GCN block (2-layer GCNConv + ReLU) on 8 Trainium2 NeuronCores.

Strategy (1D node partitioning per the sharding hint):
  - Core c owns target nodes [c*N/8, (c+1)*N/8) and every edge whose target
    (col) lands there.
  - Aggregation is reordered before the weight matmul: A_norm @ (x W) ==
    (A_norm @ x) W, so each layer gathers raw table rows, segment-sums them
    into 128-target-node blocks, then applies the dense 96x96 weights.
  - Segment-sum runs on the tensor engine: for each 128-edge chunk of the
    target-sorted edge stream, a selection matrix S[e, m] = norm[e] *
    (localcol[e] == m) is built on the vector engine with one dual-op
    tensor_scalar (is_equal then mult against an iota tile), and
    psum[128 targets, 96] += S.T @ M accumulates over the block's chunks.
    Chunks are packed densely (they may straddle block boundaries; each
    (chunk, block) segment gets its own S/matmul with zeros off-segment).
  - Self-loops of full blocks skip the gather: their table rows are a
    contiguous load and a diagonal S carries dinv^2.
  - Messages M are gathered from a fp16 [N, 96] table in DRAM with
    per-chunk indirect DMAs ([128,1] per-partition offsets - the only
    indirect form this runtime supports).
  - Layer 1 computes T2 = relu((A@x)W1 + b1) @ W2 for owned nodes
    (W2 folded in while the data is feature-major), then an 8-core
    AllGather rebuilds the full table for layer 2's gathers.
  - Layer 2 is aggregation + bias + relu only, written node-major.
"""

import os
import sys

for _p in ("/opt/trn_rl_repo", "/root/.axon_site/_ro/trn_rl_repo"):
    if os.path.isdir(_p) and _p not in sys.path:
        sys.path.insert(0, _p)

import numpy as np

import concourse.bass as bass
import concourse.bacc as bacc
import concourse.mybir as mybir
import concourse.tile as tile
from concourse import bass_utils

F16 = mybir.dt.float16
F32 = mybir.dt.float32
I32 = mybir.dt.int32

P = 128          # partitions / edges per chunk / nodes per target block
D = 96           # feature dim
NCORES = 8


def _preprocess(row, col, ew, N):
    """Bucket edges by owning core, sort by target, pack densely into
    128-edge chunks shared across cores (per-block counts padded to the
    max over cores so one SPMD program fits all eight).

    Returns per-core gather/selection metadata plus the segment schedule
    (chunk, block, first, last) that drives program generation.
    """
    npc = N // NCORES
    nblk = (npc + P - 1) // P
    nfull = npc // P          # blocks whose self-loops use the direct path

    deg = np.bincount(col, weights=ew, minlength=N) + 1.0
    dinv = (1.0 / np.sqrt(deg)).astype(np.float32)
    norm = (dinv[row] * ew * dinv[col]).astype(np.float32)
    selfn = (dinv * dinv).astype(np.float32)

    # per-core edge lists sorted by local target; self-loops only for the
    # partial tail block (full blocks handle them without a gather)
    cores = []
    counts_all = []
    nb = np.zeros(nblk, dtype=np.int64)
    tail = npc - nfull * P
    for c in range(NCORES):
        lo, hi = c * npc, (c + 1) * npc
        m = (col >= lo) & (col < hi)
        r = np.asarray(row[m], dtype=np.int64)
        cl = np.asarray(col[m] - lo, dtype=np.int64)
        w = norm[m]
        if tail:
            tn = np.arange(nfull * P, npc, dtype=np.int64)
            r = np.concatenate([r, tn + lo])
            cl = np.concatenate([cl, tn])
            w = np.concatenate([w, selfn[lo + tn]])
        order = np.argsort(cl, kind="stable")
        r, cl, w = r[order], cl[order], w[order]
        counts = np.bincount(cl // P, minlength=nblk)
        cores.append((r, cl, w))
        counts_all.append(counts)
        nb = np.maximum(nb, counts)

    L = int(nb.sum())
    nchunks = (L + P - 1) // P
    Lpad = nchunks * P
    nb_pad = nb.copy()
    nb_pad[-1] += Lpad - L      # stream tail padding charged to last block

    # block start positions in the padded stream, and the segment schedule
    starts = np.zeros(nblk + 1, dtype=np.int64)
    starts[1:] = np.cumsum(nb_pad)
    segs = []  # (chunk, block, first, last, lane_lo, lane_hi)
    for b in range(nblk):
        s, e = int(starts[b]), int(starts[b + 1])
        c0, c1 = s // P, (e - 1) // P
        for cch in range(c0, c1 + 1):
            lo_ = max(s, cch * P) - cch * P
            hi_ = min(e, (cch + 1) * P) - cch * P
            segs.append((cch, b, cch == c0, cch == c1, lo_, hi_))
    nseg = len(segs)

    rowidx = np.zeros((NCORES, P, nchunks), np.int32)
    colseg = np.zeros((NCORES, P, nseg), np.float32)
    wseg = np.zeros((NCORES, P, nseg), np.float32)
    selfw = np.zeros((NCORES, P, max(nfull, 1)), np.float32)
    for c in range(NCORES):
        r, cl, w = cores[c]
        counts = counts_all[c]
        # build the padded stream for this core
        sr = np.zeros(Lpad, np.int64)
        scl = np.zeros(Lpad, np.int64)
        sw = np.zeros(Lpad, np.float32)
        e0 = 0
        for b in range(nblk):
            n = int(counts[b])
            s = int(starts[b])
            sr[s:s + n] = r[e0:e0 + n]
            scl[s:s + n] = cl[e0:e0 + n] - b * P
            scl[s + n:int(starts[b + 1])] = 0
            sw[s:s + n] = w[e0:e0 + n]
            e0 += n
        rowidx[c] = sr.reshape(nchunks, P).T
        scl2 = scl.reshape(nchunks, P).T
        sw2 = sw.reshape(nchunks, P).T
        for si, (cch, b, _f, _l, lo_, hi_) in enumerate(segs):
            colseg[c, lo_:hi_, si] = scl2[lo_:hi_, cch]
            wseg[c, lo_:hi_, si] = sw2[lo_:hi_, cch]
        lo = c * npc
        for b in range(nfull):
            selfw[c, :, b] = selfn[lo + b * P: lo + (b + 1) * P]

    return (rowidx, colseg, wseg, selfw, segs, nchunks, npc, nblk, nfull)


def _build_program(N, npc, nblk, nfull, nchunks, segs, repeat=1):
    nseg = len(segs)
    nc = bacc.Bacc("TRN2", target_bir_lowering=False, debug=False,
                   enable_asserts=False, num_devices=NCORES)

    t1 = nc.dram_tensor("t1", [N, D], F16, kind="ExternalInput").ap()
    xo_d = nc.dram_tensor("x_own", [nblk * P, D], F16, kind="ExternalInput").ap()
    rowidx_d = nc.dram_tensor("rowidx", [P, nchunks], I32, kind="ExternalInput").ap()
    colseg_d = nc.dram_tensor("colseg", [P, nseg], F32, kind="ExternalInput").ap()
    wseg_d = nc.dram_tensor("wseg", [P, nseg], F32, kind="ExternalInput").ap()
    selfw_d = nc.dram_tensor("selfw", [P, max(nfull, 1)], F32,
                             kind="ExternalInput").ap()
    iota_d = nc.dram_tensor("iota", [P, P], F16, kind="ExternalInput").ap()
    iotac_d = nc.dram_tensor("iotac", [P, 1], F32, kind="ExternalInput").ap()
    ident_d = nc.dram_tensor("ident", [P, P], F16, kind="ExternalInput").ap()
    w1_d = nc.dram_tensor("w1", [D, D], F16, kind="ExternalInput").ap()
    w2_d = nc.dram_tensor("w2", [D, D], F16, kind="ExternalInput").ap()
    b1_d = nc.dram_tensor("b1", [D, 1], F32, kind="ExternalInput").ap()
    b2rep_d = nc.dram_tensor("b2rep", [P, D], F32, kind="ExternalInput").ap()
    out_d = nc.dram_tensor("out", [nblk * P, D], F32, kind="ExternalOutput").ap()

    with tile.TileContext(nc) as tc:
        with (
            tc.tile_pool(name="const", bufs=1) as const_pool,
            tc.tile_pool(name="meta", bufs=1) as meta_pool,
            tc.tile_pool(name="gath", bufs=8) as g_pool,
            tc.tile_pool(name="smat", bufs=4) as s_pool,
            tc.tile_pool(name="work", bufs=2) as w_pool,
            tc.tile_pool(name="own", bufs=2) as own_pool,
            tc.tile_pool(name="pagg", bufs=2, space="PSUM") as pagg_pool,
            tc.tile_pool(name="pmisc", bufs=1, space="PSUM") as pmisc_pool,
            tc.tile_pool(name="dram", bufs=1, space="DRAM") as dram_pool,
        ):
            iota_sb = const_pool.tile([P, P], F16, tag="iota")
            iotac_sb = const_pool.tile([P, 1], F32, tag="iotac")
            ident_sb = const_pool.tile([P, P], F16, tag="ident")
            w1_sb = const_pool.tile([D, D], F16, tag="w1")
            w2_sb = const_pool.tile([D, D], F16, tag="w2")
            b1_sb = const_pool.tile([D, 1], F32, tag="b1")
            b2rep_sb = const_pool.tile([P, D], F32, tag="b2rep")
            nc.sync.dma_start(iota_sb[:], iota_d[:])
            nc.sync.dma_start(iotac_sb[:], iotac_d[:])
            nc.sync.dma_start(ident_sb[:], ident_d[:])
            nc.sync.dma_start(w1_sb[:], w1_d[:])
            nc.sync.dma_start(w2_sb[:], w2_d[:])
            nc.sync.dma_start(b1_sb[:], b1_d[:])
            nc.sync.dma_start(b2rep_sb[:], b2rep_d[:])

            rowidx_sb = meta_pool.tile([P, nchunks], I32, tag="rowidx")
            colseg_sb = meta_pool.tile([P, nseg], F32, tag="colseg")
            wseg_sb = meta_pool.tile([P, nseg], F32, tag="wseg")
            selfw_sb = meta_pool.tile([P, max(nfull, 1)], F32, tag="selfw")
            nc.sync.dma_start(rowidx_sb[:], rowidx_d[:])
            nc.sync.dma_start(colseg_sb[:], colseg_d[:])
            nc.sync.dma_start(wseg_sb[:], wseg_d[:])
            nc.sync.dma_start(selfw_sb[:], selfw_d[:])

            t2_own = dram_pool.tile([nblk * P, D], F16, tag="t2own")
            t2_full = dram_pool.tile([N, D], F16, tag="t2full")

            def post_block(layer, b, psum_agg):
                rows = min(P, npc - b * P)
                if layer == 0:
                    agg_sb = w_pool.tile([P, P], F16, tag="agg_sb")
                    nc.vector.tensor_copy(agg_sb[:, :D], psum_agg[:])
                    ptr1 = pmisc_pool.tile([P, P], F16, tag="tr1")
                    nc.tensor.transpose(ptr1[:], agg_sb[:], ident_sb[:])
                    aggT_sb = w_pool.tile([D, P], F16, tag="aggT")
                    nc.scalar.activation(
                        aggT_sb[:], ptr1[:D, :],
                        mybir.ActivationFunctionType.Copy)
                    pz = pmisc_pool.tile([D, P], F32, tag="z")
                    nc.tensor.matmul(out=pz[:], lhsT=w1_sb[:], rhs=aggT_sb[:],
                                     start=True, stop=True)
                    h1T_sb = w_pool.tile([P, P], F16, tag="h1T")
                    nc.scalar.activation(
                        h1T_sb[:D, :], pz[:],
                        mybir.ActivationFunctionType.Relu,
                        bias=b1_sb[:], scale=1.0)
                    pt2 = pmisc_pool.tile([D, P], F32, tag="t2")
                    nc.tensor.matmul(out=pt2[:], lhsT=w2_sb[:],
                                     rhs=h1T_sb[:D, :], start=True, stop=True)
                    t2T_sb = w_pool.tile([P, P], F16, tag="t2T")
                    nc.vector.tensor_copy(t2T_sb[:D, :], pt2[:])
                    ptr2 = pmisc_pool.tile([P, P], F16, tag="tr2")
                    nc.tensor.transpose(ptr2[:], t2T_sb[:], ident_sb[:])
                    t2_sb = w_pool.tile([P, D], F16, tag="t2n")
                    nc.vector.tensor_copy(t2_sb[:], ptr2[:, :D])
                    nc.sync.dma_start(
                        t2_own[b * P:b * P + rows, :], t2_sb[:rows, :])
                else:
                    tmp_sb = w_pool.tile([P, D], F32, tag="tmp")
                    nc.vector.tensor_tensor(
                        out=tmp_sb[:], in0=psum_agg[:], in1=b2rep_sb[:],
                        op=mybir.AluOpType.add)
                    o_sb = w_pool.tile([P, D], F32, tag="osb")
                    nc.scalar.activation(
                        o_sb[:], tmp_sb[:],
                        mybir.ActivationFunctionType.Relu)
                    nc.sync.dma_start(out_d[b * P:(b + 1) * P, :], o_sb[:])

            for layer in [0, 1] * repeat:
                table = t1 if layer == 0 else t2_full[:]
                own_src = xo_d if layer == 0 else t2_own[:]
                psums = {}
                gbuf = None
                si = 0
                for cch in range(nchunks):
                    gbuf = g_pool.tile([P, D], F16, tag="gbuf")
                    nc.gpsimd.indirect_dma_start(
                        out=gbuf[:],
                        out_offset=None,
                        in_=table,
                        in_offset=bass.IndirectOffsetOnAxis(
                            ap=rowidx_sb[:, cch:cch + 1], axis=0),
                    )
                    while si < nseg and segs[si][0] == cch:
                        _c, b, first, last, _lo, _hi = segs[si]
                        if first:
                            psums[b] = pagg_pool.tile([P, D], F32, tag="agg")
                            if b < nfull:
                                own_sb = own_pool.tile([P, D], F16, tag="own")
                                nc.sync.dma_start(
                                    own_sb[:], own_src[b * P:(b + 1) * P, :])
                                sdiag = s_pool.tile([P, P], F16, tag="s")
                                nc.vector.tensor_scalar(
                                    out=sdiag[:],
                                    in0=iota_sb[:],
                                    scalar1=iotac_sb[:],
                                    scalar2=selfw_sb[:, b:b + 1],
                                    op0=mybir.AluOpType.is_equal,
                                    op1=mybir.AluOpType.mult,
                                )
                                nc.tensor.matmul(
                                    out=psums[b][:], lhsT=sdiag[:],
                                    rhs=own_sb[:], start=True, stop=False)
                        s_t = s_pool.tile([P, P], F16, tag="s")
                        nc.vector.tensor_scalar(
                            out=s_t[:],
                            in0=iota_sb[:],
                            scalar1=colseg_sb[:, si:si + 1],
                            scalar2=wseg_sb[:, si:si + 1],
                            op0=mybir.AluOpType.is_equal,
                            op1=mybir.AluOpType.mult,
                        )
                        nc.tensor.matmul(
                            out=psums[b][:],
                            lhsT=s_t[:],
                            rhs=gbuf[:],
                            start=(first and b >= nfull),
                            stop=last,
                        )
                        if last:
                            post_block(layer, b, psums.pop(b))
                        si += 1

                if layer == 0:
                    nc.gpsimd.collective_compute(
                        "AllGather",
                        mybir.AluOpType.bypass,
                        replica_groups=[list(range(NCORES))],
                        ins=[t2_own[:npc, :]],
                        outs=[t2_full[:]],
                    )

    nc.compile()
    return nc


_CACHE = {}


def _get_program(N, npc, nblk, nfull, nchunks, segs, repeat=1):
    key = (N, npc, nblk, nfull, nchunks, tuple(segs), repeat)
    if key not in _CACHE:
        _CACHE[key] = _build_program(N, npc, nblk, nfull, nchunks, segs,
                                     repeat=repeat)
    return _CACHE[key]


def _make_inputs(x, W1, b1, W2, b2, pre):
    rowidx, colseg, wseg, selfw, segs, nchunks, npc, nblk, nfull = pre
    t1 = np.asarray(x, np.float32).astype(np.float16)
    common = {
        "t1": t1,
        "iota": np.tile(np.arange(P, dtype=np.float16), (P, 1)),
        "iotac": np.arange(P, dtype=np.float32).reshape(P, 1),
        "ident": np.eye(P, dtype=np.float16),
        "w1": np.asarray(W1, np.float32).astype(np.float16),
        "w2": np.asarray(W2, np.float32).astype(np.float16),
        "b1": np.asarray(b1, np.float32).reshape(D, 1),
        "b2rep": np.tile(np.asarray(b2, np.float32).reshape(1, D), (P, 1)),
    }
    in_maps = []
    for c in range(NCORES):
        xo = np.zeros((nblk * P, D), np.float16)
        xo[:npc] = t1[c * npc:(c + 1) * npc]
        m = dict(common)
        m["x_own"] = xo
        m["rowidx"] = rowidx[c]
        m["colseg"] = colseg[c]
        m["wseg"] = wseg[c]
        m["selfw"] = selfw[c]
        in_maps.append(m)
    return in_maps


def kernel(x, edge_index, edge_weight, batch, W1, b1, W2, b2, **_unused):
    x = np.asarray(x, dtype=np.float32)
    edge_index = np.asarray(edge_index)
    ew = np.asarray(edge_weight, dtype=np.float32)
    N = x.shape[0]
    row = np.asarray(edge_index[0], dtype=np.int64)
    col = np.asarray(edge_index[1], dtype=np.int64)

    pre = _preprocess(row, col, ew, N)
    rowidx, colseg, wseg, selfw, segs, nchunks, npc, nblk, nfull = pre
    nc = _get_program(N, npc, nblk, nfull, nchunks, segs)
    in_maps = _make_inputs(x, W1, b1, W2, b2, pre)

    res = bass_utils.run_bass_kernel_spmd(nc, in_maps, core_ids=list(range(NCORES)))
    out = np.concatenate([res.results[c]["out"][:npc] for c in range(NCORES)],
                         axis=0)
    return out.astype(np.float32)
